# revision 1
# baseline (speedup 1.0000x reference)
"""Multi-head causal attention with RoPE on 8 TRN2 NeuronCores.

Problem: B=2, S=2048, D=1024, H=16 heads, DH=64, fp32, causal, RoPE.

Sharding (hardcoded): core c in 0..7 handles batch b = c//4 and head group
g = c%4 (heads 4g..4g+3, channels 256g..256g+256). Each core computes its
4 heads end-to-end (QKV projections, RoPE, attention, its slice of the
output projection); the host sums the 4 partial output projections per
batch. RoPE tables replicated.

Device algorithm (per core), all matmuls in float32r (full-rate PE with
~1e-3-class rounding; fp32 PSUM accumulation):
  - load x^T [D,S]; project q^T,k^T per head pair [128,2048] (channels on
    partitions) and v in natural layout [s,c] with a riding ones column
    (v_ext) for softmax denominators.
  - RoPE applied in-place on q^T/k^T: half-rotation done with 4 SBUF->SBUF
    partition-shift DMAs per chunk, then 3 DVE ops (mul/mul/add) with
    host-precomputed cos / sign-folded-sin tables.
  - attention per head in transposed-score space: S^T[k,q] tiles from
    K=64 matmuls; exp via ACT (scale=1/8 fused, no max subtraction -- scores
    are O(5), exp is safe in fp32); causal handling: k-tiles above the
    diagonal are skipped, diagonal blocks are narrowed to their live columns
    and only the true-diagonal 128x128 slice gets a triangle mask multiply;
    AV via M=65 matmuls (ones column accumulates the denominator in PSUM
    row 64); normalize: denominator broadcast by a K=1 matmul, reciprocal
    on the broadcast (all lanes), multiply.
  - output projection y = attn @ Wo^T (this core's 256 channels only).
"""
import numpy as np

B, S, D, H = 2, 2048, 1024, 16
DH = 64
NCORES = 8
P = 128
QT = 512                  # q tile (free dim)
NQT = S // QT             # 4
NKT = S // P              # 16 k tiles
NE = D // P               # 8 contraction chunks
HPC = 4                   # heads per core
C = HPC * DH              # 256 channels per core

_cache = {}


def _attention(nc, qk_pair, v_ext, mask_sb, ones_sb, attnT,
               psS, psO, ptp, normp, MM, F32, MUL, EXP):
    vhs = [v_ext.rearrange("p t (h x) -> p t h x", h=HPC)[:, :, h]
           for h in range(HPC)]
    for qt in range(NQT):
        for hp in range(2):          # head pairs, 2-way interleaved chains
            hs = (2 * hp, 2 * hp + 1)
            qhs, khs, po = {}, {}, {}
            for h in hs:
                pr, half = h // 2, (h % 2) * DH
                qhs[h] = qk_pair[("q", pr)][half:half + DH]
                khs[h] = qk_pair[("k", pr)][half:half + DH]
                po[h] = psO.tile([DH + 1, QT], F32, tag="po", name=f"po{h}")
            nkt = 4 * qt + 4
            for kt in range(nkt):
                j = kt - 4 * qt   # >= 0 on diagonal blocks
                lo = max(j, 0) * P
                for h in hs:
                    ps = psS.tile([P, QT], F32, tag="ps", name="ps")[:, lo:]
                    nc.tensor.matmul(
                        ps,
                        lhsT=khs[h][:, kt * P:(kt + 1) * P],
                        rhs=qhs[h][:, qt * QT + lo:(qt + 1) * QT])
                    pt = ptp.tile([P, QT], MM, tag="pt", name="pt")[:, lo:]
                    nc.scalar.activation(pt, ps, EXP, scale=0.125)
                    if j >= 0:
                        nc.gpsimd.tensor_tensor(pt[:, :P], pt[:, :P],
                                                mask_sb, MUL)
                    nc.tensor.matmul(po[h][:, lo:], lhsT=vhs[h][:, kt],
                                     rhs=pt,
                                     start=(kt == 0), stop=(kt == nkt - 1))
            for h in hs:
                den = normp.tile([DH + 1, QT], MM, tag="den")
                nc.vector.tensor_copy(den[DH:DH + 1], po[h][DH:DH + 1])
                bc = psO.tile([DH + 1, QT], F32, tag="po", name="bc")[:DH]
                nc.tensor.matmul(bc, lhsT=ones_sb[DH:DH + 1],
                                 rhs=den[DH:DH + 1])
                bc_sb = normp.tile([DH, QT], F32, tag="bcs")
                with nc.allow_low_precision(reason="softmax denom recip"):
                    nc.vector.reciprocal(bc_sb, bc)
                nc.vector.tensor_tensor(
                    attnT[h][:, qt * QT:(qt + 1) * QT],
                    po[h][:DH], bc_sb, MUL)


def _build():
    import concourse.bass as bass
    import concourse.mybir as mybir
    import concourse.tile as tile
    from concourse import bacc

    MM = mybir.dt.float32r
    F32 = mybir.dt.float32
    MUL = mybir.AluOpType.mult
    ADD = mybir.AluOpType.add
    EXP = mybir.ActivationFunctionType.Exp

    nc = bacc.Bacc(trn_type="TRN2", target_bir_lowering=False, debug=False,
                   enable_asserts=False)
    xT = nc.dram_tensor("xT", [D, S], MM, kind="ExternalInput").ap()
    wq_t = nc.dram_tensor("wq_t", [D, C], MM, kind="ExternalInput").ap()
    wk_t = nc.dram_tensor("wk_t", [D, C], MM, kind="ExternalInput").ap()
    wv_t = nc.dram_tensor("wv_t", [D, C], MM, kind="ExternalInput").ap()
    wo4 = nc.dram_tensor("wo4", [DH, HPC, D], MM, kind="ExternalInput").ap()
    cos2 = nc.dram_tensor("cos2", [P, S], MM, kind="ExternalInput").ap()
    sin2 = nc.dram_tensor("sin2", [P, S], MM, kind="ExternalInput").ap()
    mask1 = nc.dram_tensor("mask1", [P, P], MM, kind="ExternalInput").ap()
    onesd = nc.dram_tensor("onesd", [P, DH], MM, kind="ExternalInput").ap()
    y = nc.dram_tensor("y", [S, D], F32, kind="ExternalOutput").ap()

    with tile.TileContext(nc) as tc:
        with tc.tile_pool(name="keep", bufs=1) as keep, \
             tc.tile_pool(name="ptp", bufs=6) as ptp, \
             tc.tile_pool(name="normp", bufs=2) as normp, \
             tc.tile_pool(name="work", bufs=3) as work, \
             tc.tile_pool(name="psS", bufs=2, space="PSUM") as psS, \
             tc.tile_pool(name="psO", bufs=4, space="PSUM") as psO:

            # ---------------- persistent tiles ----------------
            qk_pair = {(w, pr): keep.tile([P, S], MM, tag=f"{w}{pr}",
                                          name=f"{w}{pr}")
                       for w in ("q", "k") for pr in range(2)}
            v_ext = keep.tile([P, NKT, HPC * (DH + 1)], MM, tag="vext")
            mask_sb = keep.tile([P, P], MM, tag="mask")
            ones_sb = keep.tile([DH + 1, DH], MM, tag="ones")
            attnT = [keep.tile([DH, S], MM, tag=f"attnT{h}", name=f"attnT{h}")
                     for h in range(HPC)]
            wo_sb = keep.tile([DH, HPC, D], MM, tag="wo")

            # ---------------- phase 1: QKV + RoPE ----------------
            with tc.tile_pool(name="ph1", bufs=2) as ph1, \
                 tc.tile_pool(name="wts", bufs=1) as wts, \
                 tc.tile_pool(name="swapp", bufs=3) as swapp, \
                 tc.tile_pool(name="psQ", bufs=2, space="PSUM") as psQ:
                wq_sb = wts.tile([P, NE, C], MM, tag="wq")
                wk_sb = wts.tile([P, NE, C], MM, tag="wk")
                wv_sb = wts.tile([P, NE, C], MM, tag="wv")
                cos_sb = wts.tile([P, S], MM, tag="cos")
                sin_sb = wts.tile([P, S], MM, tag="sin")
                xts = []
                for e in range(NE):
                    xt0 = None if e else ph1.tile([P, NE, QT], MM, tag="xt",
                                                  name="xt0")
                    if e == 0:
                        xts.append(xt0)
                    nc.sync.dma_start(
                        xts[0][:, e],
                        xT[:, 0:QT].rearrange("(o p) s -> p o s", p=P)[:, e])
                    nc.sync.dma_start(
                        wq_sb[:, e],
                        wq_t.rearrange("(o p) c -> p o c", p=P)[:, e])
                    nc.sync.dma_start(
                        wk_sb[:, e],
                        wk_t.rearrange("(o p) c -> p o c", p=P)[:, e])
                    nc.sync.dma_start(
                        wv_sb[:, e],
                        wv_t.rearrange("(o p) c -> p o c", p=P)[:, e])
                nc.sync.dma_start(cos_sb, cos2)
                nc.sync.dma_start(sin_sb, sin2)
                w_of = {"q": wq_sb, "k": wk_sb}

                for st in range(NQT):  # s quarters of 512
                    if st == 0:
                        xt = xts[0]
                    else:
                        xt = ph1.tile([P, NE, QT], MM, tag="xt")
                        for e in range(NE):
                            nc.sync.dma_start(
                                xt[:, e], xT[:, st * QT:(st + 1) * QT]
                                .rearrange("(o p) s -> p o s", p=P)[:, e])
                    if st == 2:
                        # loads needed later (attention / output projection)
                        nc.sync.dma_start(mask_sb, mask1)
                        nc.sync.dma_start(ones_sb, onesd[:DH + 1])
                        nc.sync.dma_start(
                            v_ext.rearrange("p t (h x) -> p t h x",
                                            h=HPC)[:, :, :, DH:],
                            onesd.rearrange("p (t h) -> p t h",
                                            t=NKT)[:, :, :, None])
                        nc.sync.dma_start(wo_sb, wo4)
                    sl = slice(st * QT, (st + 1) * QT)
                    # q/k projections + rope, per head pair
                    for which in ("q", "k"):
                        for pr in range(2):
                            ps = psQ.tile([P, QT], F32, tag="ps")
                            for e in range(NE):
                                nc.tensor.matmul(
                                    ps,
                                    lhsT=w_of[which][:, e, pr * P:(pr + 1) * P],
                                    rhs=xt[:, e],
                                    start=(e == 0), stop=(e == NE - 1))
                            raw = qk_pair[(which, pr)][:, sl]
                            nc.vector.tensor_copy(raw, ps)
                            sw = swapp.tile([P, QT], MM, tag="swap")
                            for a in range(4):
                                src = (a ^ 1) * 32
                                nc.sync.dma_start(sw[a * 32:(a + 1) * 32],
                                                  raw[src:src + 32])
                            nc.vector.tensor_tensor(sw, sw, sin_sb[:, sl], MUL)
                            nc.vector.tensor_tensor(raw, raw, cos_sb[:, sl], MUL)
                            nc.vector.tensor_tensor(raw, raw, sw, ADD)
                    # v projection (natural layout, strided into v_ext)
                    for sb16 in range(4):
                        kt = st * 4 + sb16
                        pv = psQ.tile([P, QT], F32, tag="ps", name="pv")[:, :C]
                        for e in range(NE):
                            nc.tensor.matmul(
                                pv,
                                lhsT=xt[:, e, sb16 * P:(sb16 + 1) * P],
                                rhs=wv_sb[:, e],
                                start=(e == 0), stop=(e == NE - 1))
                        nc.vector.tensor_copy(
                            v_ext.rearrange("p t (h x) -> p t h x",
                                            h=HPC)[:, kt, :, :DH],
                            pv.rearrange("p (h x) -> p h x", h=HPC))

            # ---------------- phase 2: attention ----------------
            _attention(nc, qk_pair, v_ext, mask_sb, ones_sb, attnT,
                       psS, psO, ptp, normp, MM, F32, MUL, EXP)

            # ---------------- phase 3: output projection ----------------
            with tc.tile_pool(name="psY", bufs=2, space="PSUM") as psY:
                for sc in range(S // P):
                    for et in range(D // QT):
                        psy = psY.tile([P, QT], F32, tag="psy")
                        for h in range(HPC):
                            nc.tensor.matmul(
                                psy,
                                lhsT=attnT[h][:, sc * P:(sc + 1) * P],
                                rhs=wo_sb[:, h, et * QT:(et + 1) * QT],
                                start=(h == 0), stop=(h == HPC - 1))
                        y_sb = work.tile([P, QT], F32, tag="ysb")
                        if (sc + et) % 2 == 0:
                            nc.vector.tensor_copy(y_sb, psy)
                        else:
                            nc.scalar.copy(y_sb, psy)
                        nc.sync.dma_start(
                            y[sc * P:(sc + 1) * P, et * QT:(et + 1) * QT],
                            y_sb)
    nc.compile()
    return nc


def _get_nc():
    if "nc" not in _cache:
        _cache["nc"] = _build()
    return _cache["nc"]


def _host_inputs(x, Wq, Wk, Wv, Wo, cos, sin):
    """Build the 8 per-core input dicts."""
    cosT = np.ascontiguousarray(cos.T).astype(np.float32)     # [DH, S]
    sinT = np.ascontiguousarray(sin.T).astype(np.float32)
    sinS = np.concatenate([-sinT[:DH // 2], sinT[DH // 2:]], axis=0)
    cos2 = np.tile(cosT, (2, 1))                              # [128, S]
    sin2 = np.tile(sinS, (2, 1))
    mask1 = (np.arange(P)[:, None] <= np.arange(P)[None, :]).astype(np.float32)
    onesd = np.ones((P, DH), np.float32)

    in_maps = []
    for c in range(NCORES):
        b, g = divmod(c, 4)
        cs = slice(C * g, C * g + C)
        in_maps.append({
            "xT": np.ascontiguousarray(x[b].T).astype(np.float32),
            "wq_t": np.ascontiguousarray(Wq[cs].T).astype(np.float32),
            "wk_t": np.ascontiguousarray(Wk[cs].T).astype(np.float32),
            "wv_t": np.ascontiguousarray(Wv[cs].T).astype(np.float32),
            "wo4": np.ascontiguousarray(
                Wo.T[cs].reshape(HPC, DH, D).transpose(1, 0, 2)
            ).astype(np.float32),
            "cos2": cos2, "sin2": sin2, "mask1": mask1, "onesd": onesd,
        })
    return in_maps


def run(x, Wq, Wk, Wv, Wo, cos, sin, mask=None, trace=False, **trace_kw):
    import os
    import time
    if not trace:
        # The axon NTFF-profile hook is not installed in all containers;
        # make sure an inherited BASS_TRACE=1 can't send us down that path.
        os.environ.setdefault("BASS_NEVER_TRACE", "1")
    from concourse.bass_utils import run_bass_kernel_spmd
    nc = _get_nc()
    in_maps = _host_inputs(np.asarray(x), np.asarray(Wq), np.asarray(Wk),
                           np.asarray(Wv), np.asarray(Wo), np.asarray(cos),
                           np.asarray(sin))
    try:
        res = run_bass_kernel_spmd(nc, in_maps, core_ids=list(range(NCORES)),
                                   trace=trace, **trace_kw)
    except Exception:
        # one retry for transient device states (e.g. NRT_EXEC_UNIT errors)
        time.sleep(15)
        res = run_bass_kernel_spmd(nc, in_maps, core_ids=list(range(NCORES)),
                                   trace=trace, **trace_kw)
    parts = [r["y"] for r in res.results]
    out = np.stack([parts[0] + parts[1] + parts[2] + parts[3],
                    parts[4] + parts[5] + parts[6] + parts[7]])
    return out.astype(np.float32), res


def kernel(x, Wq, Wk, Wv, Wo, cos, sin, mask=None, **_):
    out, _res = run(x, Wq, Wk, Wv, Wo, cos, sin, mask)
    return out



# revision 18
# speedup vs baseline: 1.1418x; 1.1418x over previous
"""Multi-head causal attention with RoPE on 8 TRN2 NeuronCores.

Problem: B=2, S=2048, D=1024, H=16 heads, DH=64, fp32 in/out, causal, RoPE.

Sharding (hardcoded): core c in 0..7 handles batch b = c//4 and head group
g = c%4 (heads 4g..4g+3, channels 256g..256g+256). Each core computes its
4 heads end-to-end (QKV projections, RoPE, attention, its slice of the
output projection); the host sums the 4 partial output projections per
batch in fp32. RoPE tables replicated.

Device algorithm (per core), fp16 operands with fp32 PSUM accumulation:
  - load x^T in prepacked [128, st, e, 512] chunks; project q^T,k^T per
    head pair [128, 2048] (channels on partitions) and v in natural layout
    with riding ones columns (v_ext) for softmax denominators.
  - q/k channel->partition order is host-permuted (per 64-channel head
    half: [0:16, 32:48, 16:32, 48:64]) so the RoPE half-rotation is an
    intra-quadrant lane swap done by one DVE stream_shuffle (no DMAs);
    then 3 DVE fp16 ops (mul/mul/add) with host-permuted cos /
    sign-folded-sin tables. Channel order cancels inside q.k dot products.
  - attention per (q-tile, head-pair) in transposed-score space S^T[k,q]:
    both heads' score tiles go to one 2-bank PSUM tile so a single ACT
    instruction computes exp for the pair (scale=1/8 fused, no max
    subtraction -- scores are O(5), safe); causal: k-tiles above the
    diagonal skipped, diagonal blocks narrowed, true-diagonal 128x128 gets
    a triangle mask multiply on GPSIMD (both heads in one op).
  - AV per pair: each head accumulates [v|ones] into a [65, 512] PSUM tile
    (denominator rides in row 64). Normalize: reciprocal on the den rows,
    K=1 matmuls broadcast the recips across 64 partitions, 2 DVE
    multiplies; the odd head's tile is stacked into attnP rows 64..127 by
    a small SBUF->SBUF DMA so the output projection can contract the pair
    with K=128 matmuls (all 4 heads in 2 accumulating matmuls per tile).
  - output projection contracts the pair-stacked attnP (K=128, two
    accumulating matmuls per tile = all 4 heads), fp16 result DMA'd per
    512-row chunk; host upcasts and sums.
"""
import numpy as np

B, S, D, H = 2, 2048, 1024, 16
DH = 64
NCORES = 8
P = 128
QT = 512                  # q tile (free dim)
NQT = S // QT             # 4
NKT = S // P              # 16 k tiles
NE = D // P               # 8 contraction chunks
HPC = 4                   # heads per core
C = HPC * DH              # 256 channels per core

# channel->partition permutation per 64-channel head half (makes rotate-half
# an intra-quadrant 16<->16 lane swap), and the matching DVE shuffle mask
PERM64 = np.r_[0:16, 32:48, 16:32, 48:64]
SHUF = list(range(16, 32)) + list(range(16))

_cache = {}


def _build():
    import concourse.mybir as mybir
    import concourse.tile as tile
    from concourse import bacc

    F16 = mybir.dt.float16
    F32 = mybir.dt.float32
    MUL = mybir.AluOpType.mult
    ADD = mybir.AluOpType.add
    EXP = mybir.ActivationFunctionType.Exp

    nc = bacc.Bacc(trn_type="TRN2", target_bir_lowering=False, debug=False,
                   enable_asserts=False)
    xTp = nc.dram_tensor("xTp", [P, NQT, NE, QT], F16, kind="ExternalInput").ap()
    wq_t = nc.dram_tensor("wq_t", [P, NE, C], F16, kind="ExternalInput").ap()
    wk_t = nc.dram_tensor("wk_t", [P, NE, C], F16, kind="ExternalInput").ap()
    wv_t = nc.dram_tensor("wv_t", [P, NE, C], F16, kind="ExternalInput").ap()
    wo_p = nc.dram_tensor("wo_p", [P, 2, D], F16, kind="ExternalInput").ap()
    cos2 = nc.dram_tensor("cos2", [P, S], F16, kind="ExternalInput").ap()
    sin2 = nc.dram_tensor("sin2", [P, S], F16, kind="ExternalInput").ap()
    mask2 = nc.dram_tensor("mask2", [P, 2, P], F16, kind="ExternalInput").ap()
    onesd = nc.dram_tensor("onesd", [P, DH], F16, kind="ExternalInput").ap()
    y = nc.dram_tensor("y", [S, D], F16, kind="ExternalOutput").ap()

    with tile.TileContext(nc) as tc:
        with tc.tile_pool(name="keep", bufs=1) as keep, \
             tc.tile_pool(name="ph1", bufs=2) as ph1, \
             tc.tile_pool(name="swp", bufs=2) as swp, \
             tc.tile_pool(name="ptp", bufs=6) as ptp, \
             tc.tile_pool(name="normp", bufs=2) as normp, \
             tc.tile_pool(name="work", bufs=2) as work, \
             tc.tile_pool(name="psA", bufs=2, space="PSUM") as psA, \
             tc.tile_pool(name="psS", bufs=2, space="PSUM") as psS, \
             tc.tile_pool(name="psO", bufs=1, space="PSUM") as psO:

            # ---------------- persistent tiles ----------------
            qk_pair = {(w, pr): keep.tile([P, S], F16, tag=f"{w}{pr}",
                                          name=f"{w}{pr}")
                       for w in ("q", "k") for pr in range(2)}
            v_ext = keep.tile([P, NKT, HPC * (DH + 1)], F16, tag="vext")
            attnP = [keep.tile([P, S], F16, tag=f"attnP{pr}",
                               name=f"attnP{pr}") for pr in range(2)]
            wo_sb = keep.tile([P, 2, D], F16, tag="wo")
            cos_sb = keep.tile([P, S], F16, tag="cos")
            sin_sb = keep.tile([P, S], F16, tag="sin")
            wq_sb = keep.tile([P, NE, C], F16, tag="wq")
            wk_sb = keep.tile([P, NE, C], F16, tag="wk")
            wv_sb = keep.tile([P, NE, C], F16, tag="wv")
            mask_sb = keep.tile([P, 2, P], F16, tag="mask")
            ones_sb = keep.tile([DH + 1, DH], F16, tag="ones")

            vx = v_ext.rearrange("p t (h x) -> p t h x", h=HPC)

            # ---------------- input loads ----------------
            nc.sync.dma_start(wq_sb, wq_t)
            nc.sync.dma_start(wk_sb, wk_t)
            nc.sync.dma_start(wv_sb, wv_t)
            nc.sync.dma_start(cos_sb, cos2)
            nc.sync.dma_start(sin_sb, sin2)
            nc.sync.dma_start(mask_sb, mask2)
            nc.sync.dma_start(ones_sb, onesd[:DH + 1])
            nc.sync.dma_start(
                vx[:, :, :, DH:],
                onesd.rearrange("p (t h) -> p t h", t=NKT)[:, :, :, None])
            nc.sync.dma_start(wo_sb, wo_p)

            w_of = {"q": wq_sb, "k": wk_sb}

            def phase1(st):
                """QKV projections + RoPE for s-tile st."""
                xt = ph1.tile([P, NE, QT], F16, tag="xt")
                nc.sync.dma_start(xt, xTp[:, st])
                sl = slice(st * QT, (st + 1) * QT)
                for which in ("q", "k"):
                    for pr in range(2):
                        ps = psA.tile([P, QT], F32, tag="ps", name="ps")
                        for e in range(NE):
                            nc.tensor.matmul(
                                ps,
                                lhsT=w_of[which][:, e, pr * P:(pr + 1) * P],
                                rhs=xt[:, e],
                                start=(e == 0), stop=(e == NE - 1))
                        raw = qk_pair[(which, pr)][:, sl]
                        nc.vector.tensor_copy(raw, ps)
                        sw = swp.tile([P, QT], F16, tag="sw")
                        nc.vector.stream_shuffle(sw, raw, SHUF)
                        nc.vector.tensor_tensor(sw, sw, sin_sb[:, sl], MUL)
                        nc.vector.tensor_tensor(raw, raw, cos_sb[:, sl], MUL)
                        nc.vector.tensor_tensor(raw, raw, sw, ADD)
                for sb in range(4):
                    kt = st * 4 + sb
                    pv = psA.tile([P, QT], F32, tag="ps", name="pv")[:, :C]
                    for e in range(NE):
                        nc.tensor.matmul(
                            pv,
                            lhsT=xt[:, e, sb * P:(sb + 1) * P],
                            rhs=wv_sb[:, e],
                            start=(e == 0), stop=(e == NE - 1))
                    nc.vector.tensor_copy(
                        vx[:, kt, :, :DH],
                        pv.rearrange("p (h x) -> p h x", h=HPC))

            def attention(qt):
                nkt = 4 * qt + 4
                sl = slice(qt * QT, (qt + 1) * QT)
                for pr in range(2):
                    poA = psO.tile([DH + 1, QT], F32, tag="poA")
                    poB = psO.tile([DH + 1, QT], F32, tag="poB")
                    qh = [qk_pair[("q", pr)][hi * DH:(hi + 1) * DH]
                          for hi in range(2)]
                    kh = [qk_pair[("k", pr)][hi * DH:(hi + 1) * DH]
                          for hi in range(2)]
                    for kt in range(nkt):
                        j = kt - 4 * qt   # >= 0 on diagonal blocks
                        lo = max(j, 0) * P
                        ps = psS.tile([P, 2, QT], F32, tag="ps", name="pss")
                        for hi in range(2):
                            nc.tensor.matmul(
                                ps[:, hi, lo:],
                                lhsT=kh[hi][:, kt * P:(kt + 1) * P],
                                rhs=qh[hi][:, qt * QT + lo:(qt + 1) * QT])
                        pt = ptp.tile([P, 2, QT], F16, tag="pt")
                        nc.scalar.activation(pt[:, :, lo:], ps[:, :, lo:],
                                             EXP, scale=0.125)
                        if j >= 0:
                            nc.gpsimd.tensor_tensor(
                                pt[:, :, lo:lo + P], pt[:, :, lo:lo + P],
                                mask_sb, MUL)
                        nc.tensor.matmul(poA[:, lo:],
                                         lhsT=vx[:, kt, 2 * pr],
                                         rhs=pt[:, 0, lo:],
                                         start=(kt == 0), stop=(kt == nkt - 1))
                        nc.tensor.matmul(poB[:, lo:],
                                         lhsT=vx[:, kt, 2 * pr + 1],
                                         rhs=pt[:, 1, lo:],
                                         start=(kt == 0), stop=(kt == nkt - 1))
                    den2 = normp.tile([DH + 1, 2, QT], F16, tag="den2")
                    with nc.allow_low_precision(reason="softmax denom recip"):
                        nc.vector.reciprocal(den2[DH:, 0], poA[DH:])
                        nc.vector.reciprocal(den2[DH:, 1], poB[DH:])
                    bcA = psA.tile([P, QT], F32, tag="ps", name="bcA")[:DH]
                    bcB = psA.tile([P, QT], F32, tag="ps", name="bcB")[:DH]
                    nc.tensor.matmul(bcA, lhsT=ones_sb[DH:], rhs=den2[DH:, 0])
                    nc.tensor.matmul(bcB, lhsT=ones_sb[DH:], rhs=den2[DH:, 1])
                    atmp = swp.tile([DH, QT], F16, tag="atmp")
                    nc.vector.tensor_tensor(attnP[pr][:DH, sl],
                                            poA[:DH], bcA, MUL)
                    nc.vector.tensor_tensor(atmp, poB[:DH], bcB, MUL)
                    nc.sync.dma_start(attnP[pr][DH:, sl], atmp)

            def proj(qt):
                """Output projection for the 512 s-rows of q-tile qt."""
                ysb = work.tile([P, 4, D], F16, tag="ysb")
                for sb in range(4):
                    sc = qt * 4 + sb
                    for et in range(2):
                        psy = psA.tile([P, QT], F32, tag="ps", name="psy")
                        for pr in range(2):
                            nc.tensor.matmul(
                                psy,
                                lhsT=attnP[pr][:, sc * P:(sc + 1) * P],
                                rhs=wo_sb[:, pr, et * QT:(et + 1) * QT],
                                start=(pr == 0), stop=(pr == 1))
                        dst = ysb[:, sb, et * QT:(et + 1) * QT]
                        if (sb + et) % 2 == 0:
                            nc.vector.tensor_copy(dst, psy)
                        else:
                            nc.scalar.copy(dst, psy)
                nc.sync.dma_start(
                    y[qt * QT:(qt + 1) * QT].rearrange("(c p) e -> p c e",
                                                       p=P), ysb)

            phase1(0)
            phase1(1)
            attention(0)
            proj(0)
            phase1(2)
            attention(1)
            proj(1)
            phase1(3)
            attention(2)
            proj(2)
            attention(3)
            proj(3)
    nc.compile()
    return nc


def _get_nc():
    if "nc" not in _cache:
        _cache["nc"] = _build()
    return _cache["nc"]


def _host_inputs(x, Wq, Wk, Wv, Wo, cos, sin):
    """Build the 8 per-core input dicts (fp16, prepacked layouts)."""
    f16 = np.float16
    ordH = (np.arange(HPC)[:, None] * DH + PERM64[None, :]).reshape(-1)  # [256]

    cosT = np.ascontiguousarray(cos.T).astype(np.float32)     # [DH, S]
    sinT = np.ascontiguousarray(sin.T).astype(np.float32)
    sinS = np.concatenate([-sinT[:DH // 2], sinT[DH // 2:]], axis=0)
    cos2 = np.tile(cosT[PERM64], (2, 1)).astype(f16)          # [128, S]
    sin2 = np.tile(sinS[PERM64], (2, 1)).astype(f16)
    mask1 = (np.arange(P)[:, None] <= np.arange(P)[None, :])
    mask2 = np.stack([mask1, mask1], axis=1).astype(f16)      # [128, 2, 128]
    onesd = np.ones((P, DH), f16)

    in_maps = []
    for c in range(NCORES):
        b, g = divmod(c, 4)
        cs = slice(C * g, C * g + C)
        xb = np.asarray(x[b], np.float32)                     # [S, D]
        xTp = xb.reshape(NQT, QT, NE, P).transpose(3, 0, 2, 1)
        wq_o = np.asarray(Wq, np.float32)[cs][ordH]           # [256, D]
        wk_o = np.asarray(Wk, np.float32)[cs][ordH]
        wv_o = np.asarray(Wv, np.float32)[cs]
        wo_o = np.asarray(Wo, np.float32).T[cs]               # [256, D]
        in_maps.append({
            "xTp": np.ascontiguousarray(xTp).astype(f16),
            "wq_t": np.ascontiguousarray(
                wq_o.T.reshape(NE, P, C).transpose(1, 0, 2)).astype(f16),
            "wk_t": np.ascontiguousarray(
                wk_o.T.reshape(NE, P, C).transpose(1, 0, 2)).astype(f16),
            "wv_t": np.ascontiguousarray(
                wv_o.T.reshape(NE, P, C).transpose(1, 0, 2)).astype(f16),
            "wo_p": np.ascontiguousarray(
                wo_o.reshape(2, P, D).transpose(1, 0, 2)).astype(f16),
            "cos2": cos2, "sin2": sin2, "mask2": mask2, "onesd": onesd,
        })
    return in_maps


def run(x, Wq, Wk, Wv, Wo, cos, sin, mask=None, trace=False, **trace_kw):
    import os
    import time
    if not trace:
        # The axon NTFF-profile hook is not installed in all containers;
        # make sure an inherited BASS_TRACE=1 can't send us down that path.
        os.environ.setdefault("BASS_NEVER_TRACE", "1")
    from concourse.bass_utils import run_bass_kernel_spmd
    nc = _get_nc()
    in_maps = _host_inputs(np.asarray(x), np.asarray(Wq), np.asarray(Wk),
                           np.asarray(Wv), np.asarray(Wo), np.asarray(cos),
                           np.asarray(sin))
    try:
        res = run_bass_kernel_spmd(nc, in_maps, core_ids=list(range(NCORES)),
                                   trace=trace, **trace_kw)
    except Exception:
        # one retry for transient device states (e.g. NRT_EXEC_UNIT errors)
        time.sleep(15)
        res = run_bass_kernel_spmd(nc, in_maps, core_ids=list(range(NCORES)),
                                   trace=trace, **trace_kw)
    parts = [r["y"].astype(np.float32) for r in res.results]
    out = np.stack([parts[0] + parts[1] + parts[2] + parts[3],
                    parts[4] + parts[5] + parts[6] + parts[7]])
    return out.astype(np.float32), res


def kernel(x, Wq, Wk, Wv, Wo, cos, sin, mask=None, **_):
    out, _res = run(x, Wq, Wk, Wv, Wo, cos, sin, mask)
    return out
